# revision 8
# baseline (speedup 1.0000x reference)
"""ALiBi causal attention (B=2, T=2048, C=1024, H=16, D=64, fp32) on 8 trn2 cores.

Sharding: core i -> batch b = i//4, head-group g = i%4 (4 heads = 256 channels).
Each core computes Q/K/V projections for its head slice, causal ALiBi attention,
and a partial output projection; host sums the 4 partials per batch.

Device dataflow (per core):
  phase 1: xT (C,T) in SBUF; QT/KT in (d,t) layout with an appended aug row
           (-slope*t/scale) so the ALiBi -slope*t term rides the matmul;
           V in (t,d) layout with an appended ones column (denominator trick).
  phase 2: per (head, 512-wide q-block, 128-wide s-tile):
           ST = KTaug^T @ QTaug  (s on partitions, t free)
           P  = exp(scale*ST + slope*s)   (ACT, per-partition bias)
           causal mask on diagonal tiles via gpsimd affine_select (fill 0)
           AV += V_aug^T @ P  -> (65, t): rows 0..63 numerator, row 64 denom
           normalize: recip(den) -> PE broadcast -> multiply
  phase 3: out_partial = attn_outT^T @ WoT  -> DMA to DRAM
"""

import math

import numpy as np

B, T, C, H, D = 2, 2048, 1024, 16, 64
HPC = 4          # heads per core
CS = HPC * D     # 256 channels per core
SCALE = D ** -0.5
NCORES = 8
NEG = -8.0e30    # pre-scale mask value; *SCALE -> -1e30 -> exp -> 0


def _slopes(n_heads: int) -> np.ndarray:
    i = np.arange(1, n_heads + 1, dtype=np.float64)
    return np.power(2.0, -8.0 * i / n_heads).astype(np.float32)


_PROGRAM = None


def _build_program():
    """Build the single-core Bass program (same program on all 8 cores)."""
    from contextlib import ExitStack

    import concourse.bass as bass
    import concourse.tile as tile
    from concourse import bacc, mybir

    f32 = mybir.dt.float32
    f32r = mybir.dt.float32r
    EXP = mybir.ActivationFunctionType.Exp

    nc = bacc.Bacc("TRN2", target_bir_lowering=False, debug=False,
                   num_devices=NCORES)
    xT = nc.declare_dram_parameter("xT", [C, T], f32, isOutput=False)
    wqT = nc.declare_dram_parameter("wqT", [C, CS], f32, isOutput=False)
    wkT = nc.declare_dram_parameter("wkT", [C, CS], f32, isOutput=False)
    wvT = nc.declare_dram_parameter("wvT", [C, CS], f32, isOutput=False)
    woT = nc.declare_dram_parameter("woT", [CS, C], f32, isOutput=False)
    qaug = nc.declare_dram_parameter("qaug", [HPC, T], f32, isOutput=False)
    bias_cols = nc.declare_dram_parameter("bias_cols", [128, HPC * 16], f32,
                                          isOutput=False)
    out = nc.declare_dram_parameter("out", [T, C], f32, isOutput=True)

    KT_C = C // 128   # 8 contraction tiles for projections
    NQT = T // 128    # 16 q/s tiles
    NQB = T // 512    # 4 q blocks

    with nc.allow_low_precision(reason="f32r is 4-byte; rounding only at PE"), \
         tile.TileContext(nc) as tc, ExitStack() as ctx:
        # ---- long-lived SBUF ----
        qk_pool = ctx.enter_context(tc.tile_pool(name="qk", bufs=1))
        qt_t = [qk_pool.tile([65, T], f32r, tag=f"qt{h}", name=f"qt{h}") for h in range(HPC)]
        kt_t = [qk_pool.tile([65, T], f32r, tag=f"kt{h}", name=f"kt{h}") for h in range(HPC)]
        v_t = qk_pool.tile([128, NQT, HPC, 65], f32r)      # V + ones column
        attn_t = [qk_pool.tile([128, T], f32r, tag=f"at{i}", name=f"at{i}") for i in range(2)]
        bias_sb = qk_pool.tile([128, HPC * 16], f32)
        ones_sb = qk_pool.tile([1, 64], f32r)
        wo_sb = qk_pool.tile([128, 2, C], f32r)

        nc.sync.dma_start(out=bias_sb[:], in_=bias_cols[:])
        nc.sync.dma_start(out=wo_sb[:, 0, :], in_=woT[0:128, :].bitcast(f32r))
        nc.sync.dma_start(out=wo_sb[:, 1, :], in_=woT[128:256, :].bitcast(f32r))
        nc.vector.memset(ones_sb[:].bitcast(f32), 1.0)
        nc.vector.memset(v_t[:].bitcast(f32), 1.0)  # ones column preset; data overwritten
        for h in range(HPC):
            nc.sync.dma_start(out=qt_t[h][64:65, :], in_=qaug[h:h + 1, :].bitcast(f32r))
            nc.vector.memset(kt_t[h][64:65, :].bitcast(f32), 1.0)

        # ---- phase 1: projections ----
        with tc.tile_pool(name="xt", bufs=1) as xt_pool, \
             tc.tile_pool(name="w", bufs=1) as w_pool, \
             tc.tile_pool(name="p1ps", bufs=4, space="PSUM") as p1ps:
            xt_sb = xt_pool.tile([128, KT_C, T], f32r)
            wq_sb = w_pool.tile([128, KT_C, CS], f32r)
            wk_sb = w_pool.tile([128, KT_C, CS], f32r)
            wv_sb = w_pool.tile([128, KT_C, CS], f32r)
            for k in range(KT_C):
                nc.sync.dma_start(out=xt_sb[:, k, :], in_=xT[k * 128:(k + 1) * 128, :].bitcast(f32r))
                nc.sync.dma_start(out=wq_sb[:, k, :], in_=wqT[k * 128:(k + 1) * 128, :].bitcast(f32r))
                nc.sync.dma_start(out=wk_sb[:, k, :], in_=wkT[k * 128:(k + 1) * 128, :].bitcast(f32r))
                nc.sync.dma_start(out=wv_sb[:, k, :], in_=wvT[k * 128:(k + 1) * 128, :].bitcast(f32r))

            # QT / KT: (d on partitions, t free)
            for (w_sb, dst) in ((wq_sb, qt_t), (wk_sb, kt_t)):
                for dt_i in range(2):          # two 128-wide d tiles
                    for tch in range(NQB):     # four 512-wide t chunks
                        ps = p1ps.tile([128, 512], f32, tag="proj")
                        for k in range(KT_C):
                            nc.tensor.matmul(
                                ps[:],
                                lhsT=w_sb[:, k, dt_i * 128:(dt_i + 1) * 128],
                                rhs=xt_sb[:, k, tch * 512:(tch + 1) * 512],
                                start=(k == 0), stop=(k == KT_C - 1),
                            )
                        for hl in range(2):
                            h = dt_i * 2 + hl
                            nc.vector.tensor_copy(
                                dst[h][0:64, tch * 512:(tch + 1) * 512],
                                ps[hl * 64:(hl + 1) * 64, :],
                            )
            # V: (t on partitions, d free)
            for st in range(NQT):
                ps = p1ps.tile([128, CS], f32, tag="vproj")
                for k in range(KT_C):
                    nc.tensor.matmul(
                        ps[:],
                        lhsT=xt_sb[:, k, st * 128:(st + 1) * 128],
                        rhs=wv_sb[:, k, :],
                        start=(k == 0), stop=(k == KT_C - 1),
                    )
                nc.vector.tensor_copy(
                    v_t[:, st, :, 0:64],
                    ps.rearrange("p (h d) -> p h d", h=HPC),
                )

        # ---- phase 2: attention ----
        with tc.tile_pool(name="stps", bufs=3, space="PSUM") as stps, \
             tc.tile_pool(name="avps", bufs=2, space="PSUM") as avps, \
             tc.tile_pool(name="bcps", bufs=2, space="PSUM") as bcps, \
             tc.tile_pool(name="pt", bufs=4) as pt_pool, \
             tc.tile_pool(name="dn", bufs=4) as dn_pool:
            for h in range(HPC):
                for qb in range(NQB):
                    n_st = 4 * qb + 4
                    av = avps.tile([65, 512], f32, tag="av")
                    for st in range(n_st):
                        sp = stps.tile([128, 512], f32, tag="st")
                        nc.tensor.matmul(
                            sp[:],
                            lhsT=kt_t[h][:, st * 128:(st + 1) * 128],
                            rhs=qt_t[h][:, qb * 512:(qb + 1) * 512],
                            start=True, stop=True,
                        )
                        pt = pt_pool.tile([128, 512], f32r, tag="pt")
                        nc.scalar.activation(
                            pt[:], sp[:], EXP,
                            bias=bias_sb[:, h * 16 + st:h * 16 + st + 1],
                            scale=SCALE,
                        )
                        if st >= 4 * qb:  # diagonal tile: causal mask
                            r = st - 4 * qb
                            nc.gpsimd.affine_select(
                                pt[:], pt[:], pattern=[[1, 512]],
                                compare_op=mybir.AluOpType.is_ge,
                                fill=0.0, base=-(128 * r), channel_multiplier=-1,
                            )
                        nc.tensor.matmul(
                            av[:],
                            lhsT=v_t[:, st, h, :],
                            rhs=pt[:],
                            start=(st == 0), stop=(st == n_st - 1),
                        )
                    den = dn_pool.tile([1, 512], f32r, tag="den")
                    nc.vector.reciprocal(den[:], av[64:65, :])
                    bc = bcps.tile([64, 512], f32, tag="bc")
                    nc.tensor.matmul(
                        bc[:], lhsT=ones_sb[:],
                        rhs=den[:], start=True, stop=True,
                    )
                    bcs = dn_pool.tile([64, 512], f32, tag="bcs")
                    nc.vector.tensor_copy(bcs[:], bc[:])
                    nc.vector.tensor_mul(
                        attn_t[h // 2][(h % 2) * 64:(h % 2) * 64 + 64,
                                       qb * 512:(qb + 1) * 512],
                        av[0:64, :], bcs[:],
                    )

        # ---- phase 3: output projection ----
        with tc.tile_pool(name="ops", bufs=4, space="PSUM") as ops, \
             tc.tile_pool(name="ot", bufs=4) as ot_pool:
            for qt_i in range(NQT):
                for chn in range(2):
                    ps = ops.tile([128, 512], f32, tag="o")
                    for kt_i in range(2):
                        nc.tensor.matmul(
                            ps[:],
                            lhsT=attn_t[kt_i][:, qt_i * 128:(qt_i + 1) * 128],
                            rhs=wo_sb[:, kt_i, chn * 512:(chn + 1) * 512],
                            start=(kt_i == 0), stop=(kt_i == 1),
                        )
                    ob = ot_pool.tile([128, 512], f32, tag="ob")
                    if chn == 0:
                        nc.vector.tensor_copy(ob[:], ps[:])
                    else:
                        nc.scalar.copy(ob[:], ps[:])
                    nc.sync.dma_start(
                        out=out[qt_i * 128:(qt_i + 1) * 128,
                                chn * 512:(chn + 1) * 512],
                        in_=ob[:],
                    )

    nc.finalize()
    return nc


def _host_inputs(x, Wq, Wk, Wv, Wo):
    """Build the 8 per-core input maps."""
    slopes = _slopes(H)
    in_maps = []
    for core in range(NCORES):
        b, g = core // 4, core % 4
        hs = slice(g * CS, (g + 1) * CS)
        sl = slopes[g * HPC:(g + 1) * HPC].astype(np.float64)
        t_idx = np.arange(T, dtype=np.float64)
        qaug = (-(sl[:, None] * t_idx[None, :]) / SCALE).astype(np.float32)
        p_idx = np.arange(128, dtype=np.float64)
        bias_cols = np.empty((128, HPC * 16), dtype=np.float32)
        for hl in range(HPC):
            for st in range(16):
                bias_cols[:, hl * 16 + st] = (sl[hl] * (st * 128 + p_idx)).astype(
                    np.float32)
        in_maps.append({
            "xT": np.ascontiguousarray(x[b].T),
            "wqT": np.ascontiguousarray(Wq[hs, :].T),
            "wkT": np.ascontiguousarray(Wk[hs, :].T),
            "wvT": np.ascontiguousarray(Wv[hs, :].T),
            "woT": np.ascontiguousarray(Wo[:, hs].T),
            "qaug": qaug,
            "bias_cols": bias_cols,
        })
    return in_maps


def get_program():
    global _PROGRAM
    if _PROGRAM is None:
        _PROGRAM = _build_program()
    return _PROGRAM


def kernel(x, Wq, Wk, Wv, Wo, _trace=False):
    from concourse.bass_utils import run_bass_kernel_spmd

    x = np.asarray(x, dtype=np.float32)
    nc = get_program()
    in_maps = _host_inputs(x, np.asarray(Wq, np.float32), np.asarray(Wk, np.float32),
                           np.asarray(Wv, np.float32), np.asarray(Wo, np.float32))
    res = run_bass_kernel_spmd(nc, in_maps, list(range(NCORES)), trace=_trace)
    kernel.last_results = res
    outs = [res.results[i]["out"] for i in range(NCORES)]
    full = np.empty((B, T, C), dtype=np.float32)
    for b in range(B):
        full[b] = outs[4 * b] + outs[4 * b + 1] + outs[4 * b + 2] + outs[4 * b + 3]
    return full
